# revision 1
# baseline (speedup 1.0000x reference)
"""Distributed attention kernel for one TRN2 chip (8 NeuronCores).

Problem: multi-head cross-attention
  B=4, TQ=512, TKV=4096, D=1024, H=8 heads (head_dim=128)

Sharding (data-parallel x tensor-parallel, per the hint):
  core c in 0..7 -> (batch b = c % 4, head-group g = c // 4)
  Each core computes heads [4g, 4g+4) for its batch (Wq/Wk/Wv column
  shards). Per-head U results are pair-exchanged (c <-> c+4) with an
  AllGather DURING the attention phase, so every core ends with all 8
  heads' U and computes its own 512-column slice of the output
  projection locally - no serialized collective tail.

Device layout (per core; everything transposed so no on-device
transposes are needed - the host passes x^T and mask^T):
  Q^T[dh, t]  = Wq_g^T x_q^T          (4 head-blocks x 8 k-chunks)
  K^T[dh, T]  = Wk_g^T x_kv^T
  V[T, dh]    = x_kv Wv_g             (from x_kv^T chunks as lhsT)
  S^T[T, t]   = K^T_h(block)^T Q^T_h  per head, 32 T-blocks
  praw        = exp(S^T/sqrt(128))    (no max-subtraction needed:
                scores are O(1) so exp cannot overflow/underflow)
  P^T         = praw * mask^T         [DVE]
  U^T[dh, t] += V_h(block)^T P^T      accumulated over T-blocks in PSUM
  den        += ones^T P^T            (PE ones-matmul = partition sum)
  U^T *= 1/max(den, tiny)             (rows with all-false mask give
                U = 0 exactly, so they stay 0 like the reference wipe)
  per head: pair AllGather of U^T, overlapped with attention
  out^T[o_own, t] = Wo_own^T U_all^T (+ bo slice), direct DMA out.

Matmul inputs are bf16 (PE 4x faster than fp32); PSUM accumulation,
softmax denominators and reciprocal stay fp32.
"""

import sys

if "/opt/trn_rl_repo" not in sys.path:
    sys.path.insert(0, "/opt/trn_rl_repo")

import numpy as np
import ml_dtypes
from contextlib import ExitStack

B, TQ, TKV, D, H = 4, 512, 4096, 1024, 8
HD = D // H            # 128 head dim
NCORES = 8
GH = H // 2            # heads per core = 4
GD = GH * HD           # 512 cols per head-group
P = 128
KC = D // P            # 8 contraction chunks
NTB = TKV // P         # 32 T-blocks
NTC = TKV // 512       # 8 T-chunks (DMA granularity)
NOB = GD // P          # 4 output blocks per core (own col half)
SCALE = float(1.0 / np.sqrt(HD))

_CACHED_NC = None


def _build_nc():
    from concourse import mybir, bacc
    from concourse.tile import TileContext

    bf = mybir.dt.bfloat16
    f32 = mybir.dt.float32
    AF = mybir.ActivationFunctionType
    OP = mybir.AluOpType

    nc = bacc.Bacc("TRN2", target_bir_lowering=False, debug=False,
                   num_devices=NCORES)

    # All inputs are pre-tiled on the host into partition-major layouts
    # so every DMA is 128 contiguous multi-KB descriptors.
    xqT = nc.dram_tensor("xqT", [P, KC, TQ], bf, kind="ExternalInput")
    xkvT = nc.dram_tensor("xkvT", [P, NTC, KC, 512], bf, kind="ExternalInput")
    maskT = nc.dram_tensor("maskT", [P, NTB, TQ], bf, kind="ExternalInput")
    Wq = nc.dram_tensor("Wq", [P, KC, GD], bf, kind="ExternalInput")
    Wk = nc.dram_tensor("Wk", [P, KC, GD], bf, kind="ExternalInput")
    Wv = nc.dram_tensor("Wv", [P, KC, GD], bf, kind="ExternalInput")
    Wo = nc.dram_tensor("Wo", [P, H, GD], bf, kind="ExternalInput")
    bq = nc.dram_tensor("bq", [GD], f32, kind="ExternalInput")
    bk = nc.dram_tensor("bk", [GD], f32, kind="ExternalInput")
    bv = nc.dram_tensor("bv", [GD], f32, kind="ExternalInput")
    bo = nc.dram_tensor("bo", [GD], f32, kind="ExternalInput")
    out = nc.dram_tensor("out", [P, NOB, TQ], bf, kind="ExternalOutput")

    with TileContext(nc) as tc:
        with ExitStack() as ctx:
            persist = ctx.enter_context(tc.tile_pool(name="persist", bufs=1))
            kvchunk = ctx.enter_context(tc.tile_pool(name="kvchunk", bufs=2))
            work = ctx.enter_context(tc.tile_pool(name="work", bufs=3))
            outp = ctx.enter_context(tc.tile_pool(name="outp", bufs=1))
            # One pool of double-bank [P, 2, TQ] psum tiles serves the
            # projections (using one half) and the attention S-tiles.
            ppool = ctx.enter_context(
                tc.tile_pool(name="ppool", bufs=2, space="PSUM"))
            upool = ctx.enter_context(
                tc.tile_pool(name="upool", bufs=2, space="PSUM"))
            dpool = ctx.enter_context(
                tc.tile_pool(name="dpool", bufs=2, space="PSUM"))
            dram = ctx.enter_context(
                tc.tile_pool(name="dram", bufs=1, space="DRAM"))

            # ---- constants / weights / biases -------------------------
            # DMA emission order matters for time-to-first-matmul: Wq+xq
            # first so the Q projection starts ~6us in, then Wk/Wv, then
            # the kv chunks; mask/Wo are only needed later.
            wq_sb = persist.tile([P, KC, GD], bf)
            xq_sb = persist.tile([P, KC, TQ], bf)

            bq_sb = persist.tile([P, GH], f32)
            bk_sb = persist.tile([P, GH], f32)
            nc.sync.dma_start(bq_sb[:], bq.ap().rearrange("(h p) -> p h", p=P))
            nc.sync.dma_start(bk_sb[:], bk.ap().rearrange("(h p) -> p h", p=P))
            bv_row = persist.tile([1, GD], f32)
            nc.sync.dma_start(bv_row[:], bv.ap().unsqueeze(0))
            bv_rep = persist.tile([P, GD], f32)
            nc.gpsimd.partition_broadcast(bv_rep[:], bv_row[:])

            ones_bf = persist.tile([P, 1], bf)
            nc.vector.memset(ones_bf[:], 1.0)


            wk_sb = persist.tile([P, KC, GD], bf)
            wv_sb = persist.tile([P, KC, GD], bf)
            kv_tiles = {}

            def load_kv_chunk(tcknk, chunked=False):
                t = kvchunk.tile([P, KC, 512], bf, name="xkv_t", tag="xkv")
                if chunked:
                    for kc in range(KC):
                        nc.sync.dma_start(t[:, kc:kc + 1, :],
                                          xkvT.ap()[:, tcknk, kc:kc + 1, :])
                else:
                    nc.sync.dma_start(t[:], xkvT.ap()[:, tcknk, :, :])
                kv_tiles[tcknk] = t

            t0c = kvchunk.tile([P, KC, 512], bf, name="xkv_t", tag="xkv")
            kv_tiles[0] = t0c
            for kc in range(KC):
                nc.sync.dma_start(wq_sb[:, kc:kc + 1, :],
                                  Wq.ap()[:, kc:kc + 1, :])
                nc.sync.dma_start(xq_sb[:, kc:kc + 1, :],
                                  xqT.ap()[:, kc:kc + 1, :])
            for kc in range(KC):
                nc.sync.dma_start(wk_sb[:, kc:kc + 1, :],
                                  Wk.ap()[:, kc:kc + 1, :])
                nc.sync.dma_start(t0c[:, kc:kc + 1, :],
                                  xkvT.ap()[:, 0, kc:kc + 1, :])
            nc.sync.dma_start(wv_sb[:], Wv.ap())

            # ---- Q^T = Wq_g^T x_q^T  (+bq) ----------------------------
            # kc-major: all 4 head-blocks accumulate in parallel PSUM
            # banks so the first matmul only needs the kc=0 DMA slices,
            # not the whole 2 MB of Wq+xq.
            qt_sb = persist.tile([P, GH, TQ], bf)
            q_ps = [ppool.tile([P, 2, TQ], f32, name="q_ps", tag="big")
                    for _ in range(2)]
            for kc in range(KC):
                for db in range(GH):
                    nc.tensor.matmul(q_ps[db // 2][:, db % 2, :],
                                     wq_sb[:, kc, db * P:(db + 1) * P],
                                     xq_sb[:, kc, :],
                                     start=(kc == 0), stop=(kc == KC - 1))
            for db in range(GH):
                nc.vector.tensor_tensor(
                    qt_sb[:, db, :], q_ps[db // 2][:, db % 2, :],
                    bq_sb[:, db:db + 1].to_broadcast([P, TQ]), OP.add)

            # ---- K^T and V over T-chunks ------------------------------
            kt_sb = persist.tile([P, GH, TKV], bf)
            v_sb = persist.tile([P, NTB, GD], bf)
            mask_sb = persist.tile([P, NTB, TQ], bf)
            bo_sb = persist.tile([P, NOB], f32)
            wo_sb = persist.tile([P, H, GD], bf)
            for tcknk in range(NTC):
                if tcknk + 1 < NTC:
                    load_kv_chunk(tcknk + 1, chunked=True)
                xkv_t = kv_tiles.pop(tcknk)
                if tcknk == 1:
                    # queue the bulk "later-phase" loads behind chunks 0-1
                    nc.sync.dma_start(mask_sb[:], maskT.ap())
                    nc.sync.dma_start(wo_sb[:], Wo.ap())
                    nc.sync.dma_start(
                        bo_sb[:], bo.ap().rearrange("(ob p) -> p ob", p=P))
                for db in range(GH):
                    ps = ppool.tile([P, 2, 512], f32, name="proj_ps",
                                    tag="big")[:, 0, :]
                    for kc in range(KC):
                        nc.tensor.matmul(ps[:], wk_sb[:, kc, db * P:(db + 1) * P],
                                         xkv_t[:, kc, :],
                                         start=(kc == 0), stop=(kc == KC - 1))
                    nc.vector.tensor_tensor(
                        kt_sb[:, db, tcknk * 512:(tcknk + 1) * 512], ps[:],
                        bk_sb[:, db:db + 1].to_broadcast([P, 512]), OP.add)
                for tb in range(4):
                    ps = ppool.tile([P, 2, 512], f32, name="proj_ps",
                                    tag="big")[:, 0, :]
                    for kc in range(KC):
                        nc.tensor.matmul(ps[:],
                                         xkv_t[:, kc, tb * P:(tb + 1) * P],
                                         wv_sb[:, kc, :],
                                         start=(kc == 0), stop=(kc == KC - 1))
                    nc.vector.tensor_tensor(
                        v_sb[:, tcknk * 4 + tb, :], ps[:], bv_rep[:], OP.add)

            # ---- attention, flattened double-step loop ----------------
            # Two T-blocks per step: two S-matmuls fill the two banks of
            # one [P, 2, TQ] psum tile, then ONE wide exp (ACT per-op
            # overhead amortized below the PE pace), one wide mask-mult
            # (bf16, feeds U) and one wide mask-mult to fp8 (feeds den).
            ut_sb = persist.tile([P, GH, TQ], bf)
            u_all = persist.tile([P, 2, GH, TQ], bf)
            cc_in012 = dram.tile([3, P, TQ], bf, name="cc_in012")
            cc_out012 = dram.tile([2, 3, P, TQ], bf, name="cc_out012")
            cc_in3 = dram.tile([P, TQ], bf, name="cc_in3")
            cc_out3 = dram.tile([2, P, TQ], bf, name="cc_out3")
            RG = [[0, 1], [2, 3], [4, 5], [6, 7]]

            NDS = GH * NTB // 2
            s_tiles = {}
            u_tiles = [None] * GH
            den_tiles = [None] * GH
            SPRE = 2  # double-step prefetch depth

            def s2_mm(ds):
                t2 = ppool.tile([P, 2, TQ], f32, name="s2_ps", tag="big")
                for k in range(2):
                    h, j = divmod(ds * 2 + k, NTB)
                    nc.tensor.matmul(t2[:, k, :],
                                     kt_sb[:, h, j * P:(j + 1) * P],
                                     qt_sb[:, h, :], start=True, stop=True)
                return t2

            deferred = {}
            fstate = {}

            def fin_max(h):
                den_sf = work.tile([1, TQ], f32, tag="den_sf", bufs=2)
                nc.vector.tensor_scalar(den_sf[:], den_tiles[h][:],
                                        1e-30, None, OP.max)
                fstate[h] = [den_sf]

            def fin_recip(h):
                den_sf, = fstate[h]
                recip = work.tile([1, TQ], f32, tag="recip", bufs=2)
                nc.vector.reciprocal(recip[:], den_sf[:])
                recip_rep = work.tile([P, TQ], f32, tag="recip_rep", bufs=2)
                nc.gpsimd.partition_broadcast(recip_rep[:], recip[:])
                fstate[h] = [recip_rep]

            def fin_scale_exch(h):
                recip_rep, = fstate.pop(h)
                nc.vector.tensor_tensor(ut_sb[:, h, :], u_tiles[h][:],
                                        recip_rep[:], OP.mult)
                # pair-exchange U while attention continues: heads 0-2
                # in one batched AllGather launched at head 2, head 3 in a
                # final small one overlapped with the stage-A out-proj.
                if h < 3:
                    nc.sync.dma_start(cc_in012[h], ut_sb[:, h, :])
                else:
                    nc.sync.dma_start(cc_in3[:], ut_sb[:, h, :])
                if h == 2:
                    nc.gpsimd.collective_compute(
                        "AllGather", OP.bypass, replica_groups=RG,
                        ins=[cc_in012.opt()], outs=[cc_out012.opt()])
                    for r in range(2):
                        nc.sync.dma_start(
                            u_all[:, r, 0:3, :],
                            cc_out012[r].rearrange("h p t -> p h t"))
                elif h == 3:
                    nc.gpsimd.collective_compute(
                        "AllGather", OP.bypass, replica_groups=RG,
                        ins=[cc_in3.opt()], outs=[cc_out3.opt()])
                    nc.sync.dma_start(
                        u_all[:, :, 3, :],
                        cc_out3[:].rearrange("r p t -> p r t"))

            p_tiles = {}
            for pre in range(SPRE):
                s_tiles[pre] = s2_mm(pre)
            # U/den run one double-step behind exp/mask so their moving
            # operand is always ready when they reach the PE queue head.
            for it in range(NDS + 1):
                if it < NDS:
                    h, j0 = divmod(it * 2, NTB)
                    t2 = s_tiles.pop(it)
                    praw = work.tile([P, 2, TQ], bf, tag="praw", bufs=2)
                    nc.scalar.activation(praw[:], t2[:], AF.Exp, scale=SCALE)
                    p_t = work.tile([P, 2, TQ], bf, tag="p_t", bufs=3)
                    nc.vector.tensor_tensor(p_t[:], praw[:],
                                            mask_sb[:, j0:j0 + 2, :], OP.mult)
                    p_tiles[it] = p_t
                if it >= 1:
                    dsu = it - 1
                    h, j0 = divmod(dsu * 2, NTB)
                    if j0 == 0:
                        u_tiles[h] = upool.tile([P, TQ], f32, name="u_ps",
                                                tag="u_ps")
                        den_tiles[h] = dpool.tile([1, TQ], f32, name="den_ps",
                                                  tag="den_ps")
                    p_t = p_tiles.pop(dsu)
                    # den first: it carries the p_t semaphore wait, and its
                    # 1-column weight load is free to expose; the U matmuls
                    # then prefetch their V-block weights during its streams.
                    for k in range(2):
                        j = j0 + k
                        nc.tensor.matmul(den_tiles[h][:], ones_bf[:],
                                         p_t[:, k, :],
                                         start=(j == 0), stop=(j == NTB - 1))
                    for k in range(2):
                        j = j0 + k
                        nc.tensor.matmul(u_tiles[h][:],
                                         v_sb[:, j, h * P:(h + 1) * P],
                                         p_t[:, k, :],
                                         start=(j == 0), stop=(j == NTB - 1))
                if it < NDS and it + SPRE < NDS:
                    s_tiles[it + SPRE] = s2_mm(it + SPRE)
                for fn in deferred.pop(it, []):
                    fn()
                if it >= 1:
                    dsu = it - 1
                    h, j0 = divmod(dsu * 2, NTB)
                    if j0 + 2 == NTB:
                        # Defer the reciprocal chain a few double-steps
                        # into the next head so its DVE ops don't delay
                        # the mask-mults that feed the U matmuls.
                        deferred.setdefault(it + 2, []).append(
                            lambda h=h: fin_max(h))
                        deferred.setdefault(it + 3, []).append(
                            lambda h=h: fin_recip(h))
                        deferred.setdefault(it + 4, []).append(
                            lambda h=h: fin_scale_exch(h))

            for ds_late in sorted(deferred):
                for fn in deferred.pop(ds_late):
                    fn()

            # ---- out cols [g*512,(g+1)*512) = Wo_own^T U_all (+bo) ----
            # Stage A: heads 0-2 of both ranks (AG012 already landed)
            # overlaps the in-flight AG3; stage B adds heads 3/7.
            o_sb = outp.tile([P, NOB, TQ], bf)
            o_ps = [ppool.tile([P, 2, TQ], f32, name="o_ps", tag="big")
                    for _ in range(2)]

            def ops(ob):
                return o_ps[ob // 2][:, ob % 2, :]

            for ob in range(NOB):
                for idx, hh in enumerate((0, 1, 2, 4, 5, 6)):
                    r, lh = divmod(hh, GH)
                    nc.tensor.matmul(ops(ob),
                                     wo_sb[:, hh, ob * P:(ob + 1) * P],
                                     u_all[:, r, lh, :],
                                     start=(idx == 0), stop=False)
            for ob in range(NOB):
                for hh in (3, 7):
                    r, lh = divmod(hh, GH)
                    nc.tensor.matmul(ops(ob),
                                     wo_sb[:, hh, ob * P:(ob + 1) * P],
                                     u_all[:, r, lh, :],
                                     start=False, stop=(hh == 7))
                nc.vector.tensor_tensor(
                    o_sb[:, ob, :], ops(ob),
                    bo_sb[:, ob:ob + 1].to_broadcast([P, TQ]), OP.add)
                nc.sync.dma_start(out.ap()[:, ob:ob + 1, :],
                                  o_sb[:, ob:ob + 1, :])

    nc.finalize()
    return nc


def _ptile(a2d, inner):
    """[R, C] row-major -> [P, R//P, C] partition-major, contiguous."""
    r, c = a2d.shape
    return np.ascontiguousarray(
        a2d.reshape(r // P, P, c).transpose(1, 0, 2)).astype(inner)


def _shard_inputs(inputs_q, inputs_kv, attention_mask, Wq, bq, Wk, bk, Wv, bv,
                  Wo, bo):
    bf16 = ml_dtypes.bfloat16
    f32 = np.float32

    xqT = [_ptile(inputs_q[b].T, bf16) for b in range(B)]         # [P,KC,TQ]
    xkvT = [_ptile(inputs_kv[b].T, bf16)                          # [P,NTC,KC,512]
            .reshape(P, KC, NTC, 512).transpose(0, 2, 1, 3).copy()
            for b in range(B)]
    maskT = [_ptile(attention_mask[b].T.astype(np.float32), bf16)  # [P,NTB,TQ]
             for b in range(B)]
    in_maps = []
    for c in range(NCORES):
        b, g = c // 2, c % 2  # pair = (2b, 2b+1)
        sl = slice(g * GD, (g + 1) * GD)
        in_maps.append({
            "xqT": xqT[b],
            "xkvT": xkvT[b],
            "maskT": maskT[b],
            "Wq": _ptile(np.ascontiguousarray(Wq[:, sl]), bf16),
            "Wk": _ptile(np.ascontiguousarray(Wk[:, sl]), bf16),
            "Wv": _ptile(np.ascontiguousarray(Wv[:, sl]), bf16),
            # all head rows x own col half, [P, H, GD] bf16
            "Wo": _ptile(np.ascontiguousarray(Wo[:, sl]), bf16),
            "bq": np.ascontiguousarray(bq[sl]).astype(f32),
            "bk": np.ascontiguousarray(bk[sl]).astype(f32),
            "bv": np.ascontiguousarray(bv[sl]).astype(f32),
            "bo": np.ascontiguousarray(bo[sl]).astype(f32),
        })
    return in_maps


def kernel(_trace=False, **inputs):
    global _CACHED_NC
    from concourse import bass_utils

    arrs = {k: np.asarray(v) for k, v in inputs.items()}
    in_maps = _shard_inputs(**arrs)

    if _CACHED_NC is None:
        _CACHED_NC = _build_nc()

    res = bass_utils.run_bass_kernel_spmd(
        _CACHED_NC, in_maps, core_ids=list(range(NCORES)), trace=_trace)

    full = np.empty((B, TQ, D), np.float32)
    for c in range(NCORES):
        b, g = c // 2, c % 2
        o = res.results[c]["out"]  # [P, NOB, TQ] bf16, o-col = ob*128+p
        full[b][:, g * GD:(g + 1) * GD] = (
            o.transpose(2, 1, 0).reshape(TQ, GD).astype(np.float32))
    if _trace:
        return full, res
    return full



# revision 8
# speedup vs baseline: 1.0518x; 1.0518x over previous
"""Distributed attention kernel for one TRN2 chip (8 NeuronCores).

Problem: multi-head cross-attention
  B=4, TQ=512, TKV=4096, D=1024, H=8 heads (head_dim=128)

Sharding (data-parallel x tensor-parallel, per the hint):
  core c in 0..7 -> (batch b = c // 2, head-group g = c % 2)
  Each core computes heads [4g, 4g+4) for its batch (Wq/Wk/Wv column
  shards), pair-exchanges normalized U with core (b, 1-g) via AllGather,
  and computes its own 512-column slice of the output projection.

Pipeline structure (the key perf idea vs a phase-separated kernel):
  The attention math for T-chunk tc-1 is interleaved instruction-by-
  instruction with the K/V projection matmuls of T-chunk tc, so the
  ACT-engine exp and DVE mask/denominator work run entirely under the
  PE-bound projection stream.  The softmax denominator is accumulated
  on DVE (acc += p per double-step, bf16) instead of PE ones-matmuls,
  cutting ~27us of PE streaming.  PSUM budget is exactly 8 banks:
  4 U accumulators (whole kernel) + 2 S-tile banks (single-buffered,
  WAR hidden by the interleave) + 2 projection banks (double-buffered).

  Per-core PE work is the FLOP-minimal 13.96 GFLOP = ~178us of bf16
  streaming; everything else hides under it.

Tail: per-head finalize (den ones-matmul -> clamp -> fast-reciprocal ->
  PE ones-broadcast -> scale) pipelines into the attention drain of the
  last chunk; a single pair AllGather of all 4 heads' U^T follows, with
  a dummy warmup AllGather at kernel start so the CC stream is hot.
  Out-proj reads gathered halves slice-by-slice so its matmuls start
  as the readback lands.
"""

import sys

if "/opt/trn_rl_repo" not in sys.path:
    sys.path.insert(0, "/opt/trn_rl_repo")

import numpy as np
import ml_dtypes
from contextlib import ExitStack

B, TQ, TKV, D, H = 4, 512, 4096, 1024, 8
HD = D // H            # 128 head dim
NCORES = 8
GH = H // 2            # heads per core = 4
GD = GH * HD           # 512 cols per head-group
P = 128
KC = D // P            # 8 contraction chunks
NTB = TKV // P         # 32 T-blocks
NTC = TKV // 512       # 8 T-chunks
NOB = GD // P          # 4 output blocks per core (own col half)
SCALE = float(1.0 / np.sqrt(HD))

_CACHED_NC = None


def _build_nc():
    from concourse import mybir, bacc
    from concourse.tile import TileContext

    bf = mybir.dt.bfloat16
    f32 = mybir.dt.float32
    AF = mybir.ActivationFunctionType
    OP = mybir.AluOpType

    nc = bacc.Bacc("TRN2", target_bir_lowering=False, debug=False,
                   num_devices=NCORES)

    # Host pre-tiles everything partition-major so DMAs are 128 x multi-KB
    # contiguous descriptors.
    xqT = nc.dram_tensor("xqT", [P, KC, TQ], bf, kind="ExternalInput")
    xkvT = nc.dram_tensor("xkvT", [P, NTC, KC, 512], bf, kind="ExternalInput")
    maskT = nc.dram_tensor("maskT", [P, NTB, TQ], bf, kind="ExternalInput")
    Wq = nc.dram_tensor("Wq", [P, KC, GD], bf, kind="ExternalInput")
    Wk = nc.dram_tensor("Wk", [P, KC, GD], bf, kind="ExternalInput")
    Wv = nc.dram_tensor("Wv", [P, KC, GD], bf, kind="ExternalInput")
    Wo = nc.dram_tensor("Wo", [P, H, GD], bf, kind="ExternalInput")
    bq = nc.dram_tensor("bq", [P, GH], f32, kind="ExternalInput")
    bk = nc.dram_tensor("bk", [P, GH], f32, kind="ExternalInput")
    bv = nc.dram_tensor("bv", [GD], f32, kind="ExternalInput")
    bo = nc.dram_tensor("bo", [P, NOB], f32, kind="ExternalInput")
    out = nc.dram_tensor("out", [P, NOB, TQ], bf, kind="ExternalOutput")

    RG = [[0, 1], [2, 3], [4, 5], [6, 7]]

    with TileContext(nc) as tc:
        with ExitStack() as ctx:
            persist = ctx.enter_context(tc.tile_pool(name="persist", bufs=1))
            kvchunk = ctx.enter_context(tc.tile_pool(name="kvchunk", bufs=2))
            work = ctx.enter_context(tc.tile_pool(name="work", bufs=3))
            # PSUM: exactly 8 banks.
            upool = ctx.enter_context(
                tc.tile_pool(name="upool", bufs=4, space="PSUM"))   # 4 banks
            spool = ctx.enter_context(
                tc.tile_pool(name="spool", bufs=1, space="PSUM"))   # 2 banks
            projp = ctx.enter_context(
                tc.tile_pool(name="projp", bufs=2, space="PSUM"))   # 2 banks
            dram = ctx.enter_context(
                tc.tile_pool(name="dram", bufs=1, space="DRAM"))

            # ---- startup DMAs on two HWDGE queues (sync + scalar) ------
            wq_sb = persist.tile([P, KC, GD], bf)
            xq_sb = persist.tile([P, KC, TQ], bf)
            for kc in range(KC):
                nc.sync.dma_start(wq_sb[:, kc:kc + 1, :], Wq.ap()[:, kc:kc + 1, :])
                nc.scalar.dma_start(xq_sb[:, kc:kc + 1, :], xqT.ap()[:, kc:kc + 1, :])

            wk_sb = persist.tile([P, KC, GD], bf)
            wv_sb = persist.tile([P, KC, GD], bf)
            nc.sync.dma_start(wk_sb[:], Wk.ap())
            kv_tiles = {}
            t0c = kvchunk.tile([P, KC, 512], bf, name="xkv_t", tag="xkv")
            nc.scalar.dma_start(t0c[:], xkvT.ap()[:, 0, :, :])
            kv_tiles[0] = t0c
            nc.sync.dma_start(wv_sb[:], Wv.ap())

            bq_sb = persist.tile([P, GH], f32)
            bk_sb = persist.tile([P, GH], f32)
            bo_sb = persist.tile([P, NOB], f32)
            bv_row = persist.tile([1, GD], f32)
            nc.sync.dma_start(bq_sb[:], bq.ap())
            nc.sync.dma_start(bk_sb[:], bk.ap())
            nc.sync.dma_start(bo_sb[:], bo.ap())
            nc.sync.dma_start(bv_row[:], bv.ap().unsqueeze(0))
            bv_rep = persist.tile([P, GD], f32)
            nc.gpsimd.partition_broadcast(bv_rep[:], bv_row[:])

            ones_bf = persist.tile([P, 1], bf)
            nc.vector.memset(ones_bf[:], 1.0)

            # den accumulators (bf16; positive sums, relative errors wash)
            acc = [persist.tile([P, 2, TQ], bf, name=f"acc{h}") for h in range(GH)]
            for h in range(GH):
                nc.vector.memset(acc[h][:], 0.0)

            # warm up the CC stream so the tail AllGather has no cold-start
            warm_in = dram.tile([1, 64], bf, name="warm_in")
            warm_out = dram.tile([2, 64], bf, name="warm_out")
            nc.gpsimd.collective_compute(
                "AllGather", mybir.AluOpType.bypass, replica_groups=RG,
                ins=[warm_in.opt()], outs=[warm_out.opt()])

            # ---- Q^T = Wq_g^T x_q^T (+bq), kc-major over 4 upool banks --
            qt_sb = persist.tile([P, GH, TQ], bf)
            q_ps = [upool.tile([P, TQ], f32, name="q_ps", tag="u")
                    for _ in range(GH)]
            for kc in range(KC):
                for db in range(GH):
                    nc.tensor.matmul(q_ps[db][:],
                                     wq_sb[:, kc, db * P:(db + 1) * P],
                                     xq_sb[:, kc, :],
                                     start=(kc == 0), stop=(kc == KC - 1))
            for db in range(GH):
                nc.scalar.activation(qt_sb[:, db, :], q_ps[db][:],
                                     AF.Identity, bias=bq_sb[:, db:db + 1])

            # ---- persistent SBUF for the streamed phase ----------------
            kt_sb = persist.tile([P, GH, TKV], bf)
            v_sb = persist.tile([P, NTB, GD], bf)
            mask_q = [persist.tile([P, 8, TQ], bf, name=f"mask{q}")
                      for q in range(4)]
            wo_sb = persist.tile([P, H, GD], bf)
            ut_sb = persist.tile([P, GH, TQ], bf)
            u_all = persist.tile([P, 2, GH, TQ], bf)
            o_sb = persist.tile([P, NOB, TQ], bf)
            cc_in = dram.tile([P, GH, TQ], bf, name="cc_in")
            cc_out = dram.tile([2, P, GH, TQ], bf, name="cc_out")

            u_ps = [None] * GH
            den_ps = [None] * GH
            rc_state = {}

            # attention double-step state machine (lag-2 U behind S)
            s_tiles = {}
            p_tiles = {}

            def emit_S(ds):
                h, jp = divmod(ds, NTB // 2)
                j0 = 2 * jp
                t2 = spool.tile([P, 2, TQ], f32, name="s2_ps", tag="s")
                for k in range(2):
                    j = j0 + k
                    nc.tensor.matmul(t2[:, k, :],
                                     kt_sb[:, h, j * P:(j + 1) * P],
                                     qt_sb[:, h, :], start=True, stop=True)
                s_tiles[ds] = t2

            def emit_exp_mask(ds):
                h, jp = divmod(ds, NTB // 2)
                j0 = 2 * jp
                t2 = s_tiles.pop(ds)
                p_t = work.tile([P, 2, TQ], bf, tag="p_t", bufs=3)
                nc.scalar.activation(p_t[:], t2[:], AF.Exp, scale=SCALE)
                q, r0 = divmod(j0, 8)
                nc.vector.tensor_tensor(p_t[:], p_t[:],
                                        mask_q[q][:, r0:r0 + 2, :], OP.mult)
                nc.vector.tensor_tensor(acc[h][:], acc[h][:], p_t[:], OP.add)
                p_tiles[ds] = p_t

            def emit_U(ds):
                h, jp = divmod(ds, NTB // 2)
                j0 = 2 * jp
                if jp == 0:
                    u_ps[h] = upool.tile([P, TQ], f32, name="u_ps", tag="u")
                p_t = p_tiles.pop(ds)
                for k in range(2):
                    j = j0 + k
                    nc.tensor.matmul(u_ps[h][:],
                                     v_sb[:, j, h * P:(h + 1) * P],
                                     p_t[:, k, :],
                                     start=(j == 0), stop=(j == NTB - 1))

            # per-head finalize: den -> recip -> broadcast -> scale -> send
            def emit_fin(h):
                dps = projp.tile([P, TQ], f32, name="den_ps", tag="proj")
                for k in range(2):
                    nc.tensor.matmul(dps[0:1, :], ones_bf[:], acc[h][:, k, :],
                                     start=(k == 0), stop=(k == 1))
                den_cl = work.tile([1, TQ], f32, tag="den_cl", bufs=2)
                nc.vector.tensor_scalar(den_cl[:], dps[0:1, :], 1e-20, None,
                                        OP.max)
                recip = work.tile([1, TQ], f32, tag="recip", bufs=2)
                nc.vector.reciprocal_approx_fast(out=recip[:], in_=den_cl[:])
                rc = work.tile([P, TQ], f32, tag="rc_rep", bufs=2)
                nc.gpsimd.partition_broadcast(rc[:], recip[:])
                nc.vector.tensor_tensor(ut_sb[:, h, :], u_ps[h][:],
                                        rc[:], OP.mult)
                nc.sync.dma_start(cc_in[:, h, :], ut_sb[:, h, :])

            # ---- main streamed loop: proj(tc) interleaved with attn(tc-1)
            # per chunk: 8 proj groups (K db0-3, V tb0-3) and 8 double-steps
            # of the previous chunk's attention, round-robined so the PE
            # stream is dense and single-buffered S-psum never stalls.
            NDS = GH * NTB // 2   # 64 double-steps total
            # double-step visit order: chunk-major, head-minor
            ds_order = []
            for tcnk in range(NTC):
                for h in range(GH):
                    for pz in range(2):
                        ds_order.append(h * (NTB // 2) + tcnk * 2 + pz)
            s_q = list(ds_order)        # S-emission queue
            em_q = list(ds_order)       # exp/mask queue
            u_q = list(ds_order)        # U queue
            n_s = n_em = n_u = 0

            def pump(ns, nem, nu):
                # exp first: the next S matmul recycles the single-buffered
                # S psum slot, so its WAR must see the exp reader emitted.
                nonlocal n_s, n_em, n_u
                while n_em < nem and em_q:
                    emit_exp_mask(em_q.pop(0)); n_em += 1
                while n_s < ns and s_q:
                    emit_S(s_q.pop(0)); n_s += 1
                while n_u < nu and u_q:
                    emit_U(u_q.pop(0)); n_u += 1

            for tcnk in range(NTC):
                # stream next chunk + the mask quarter needed one chunk out
                if tcnk + 1 < NTC:
                    t = kvchunk.tile([P, KC, 512], bf, name="xkv_t", tag="xkv")
                    nc.sync.dma_start(t[:], xkvT.ap()[:, tcnk + 1, :, :])
                    kv_tiles[tcnk + 1] = t
                if tcnk % 2 == 0:
                    q = tcnk // 2
                    nc.sync.dma_start(mask_q[q][:], maskT.ap()[:, 8 * q:8 * q + 8, :])
                if tcnk == 2:
                    nc.sync.dma_start(wo_sb[:], Wo.ap())
                xkv_t = kv_tiles.pop(tcnk)

                # 8 proj groups interleaved with the attn pipeline of the
                # PREVIOUS chunk (its K/V tiles are fully in SBUF); U lags
                # the S matmuls by 2 double-steps so exp+mask always clear
                # the DVE/ACT queues before the PE reaches the U matmuls.
                base = (tcnk - 1) * 8
                for i in range(8):
                    ps = projp.tile([P, 512], f32, name="proj_ps", tag="proj")
                    if i < 4:
                        db = i
                        for kc in range(KC):
                            nc.tensor.matmul(ps[:],
                                             wk_sb[:, kc, db * P:(db + 1) * P],
                                             xkv_t[:, kc, :],
                                             start=(kc == 0), stop=(kc == KC - 1))
                        nc.scalar.activation(
                            kt_sb[:, db, tcnk * 512:(tcnk + 1) * 512], ps[:],
                            AF.Identity, bias=bk_sb[:, db:db + 1])
                    else:
                        tb = i - 4
                        for kc in range(KC):
                            nc.tensor.matmul(ps[:],
                                             xkv_t[:, kc, tb * P:(tb + 1) * P],
                                             wv_sb[:, kc, :],
                                             start=(kc == 0), stop=(kc == KC - 1))
                        nc.vector.tensor_tensor(
                            v_sb[:, tcnk * 4 + tb, :], ps[:], bv_rep[:], OP.add)
                    # pump the attention pipeline: one ds per group slot
                    tgt = base + i + 1
                    pump(tgt, tgt - 1, tgt - 2)

            # drain: remaining double-steps of chunk 7, finalizing each head
            # as soon as its last U accumulation is emitted (the finalize
            # chain fills the PE idle slots of the ACT-paced drain).
            fin_done = 0
            while s_q or em_q or u_q:
                pump(n_s + 1, n_em + 1, n_u + 1)
                # in ds_order, head h's last U is at position 56 + 2h + 1
                while fin_done < GH and n_u >= 58 + 2 * fin_done:
                    emit_fin(fin_done)
                    fin_done += 1
            while fin_done < GH:
                emit_fin(fin_done)
                fin_done += 1

            # ---- pair AllGather of U^T, then out-proj ------------------
            nc.gpsimd.collective_compute(
                "AllGather", mybir.AluOpType.bypass, replica_groups=RG,
                ins=[cc_in.opt()], outs=[cc_out.opt()])

            o_ps = [upool.tile([P, TQ], f32, name="o_ps", tag="u")
                    for _ in range(NOB)]
            # readback slice-by-slice (r, lh) so out-proj starts immediately
            for hh in range(H):
                r, lh = divmod(hh, GH)
                nc.sync.dma_start(u_all[:, r, lh, :], cc_out[r, :, lh, :])
                for ob in range(NOB):
                    nc.tensor.matmul(o_ps[ob][:],
                                     wo_sb[:, hh, ob * P:(ob + 1) * P],
                                     u_all[:, r, lh, :],
                                     start=(hh == 0), stop=(hh == H - 1))
            for ob in range(NOB):
                nc.scalar.activation(o_sb[:, ob, :], o_ps[ob][:],
                                     AF.Identity, bias=bo_sb[:, ob:ob + 1])
                nc.sync.dma_start(out.ap()[:, ob:ob + 1, :],
                                  o_sb[:, ob:ob + 1, :])

    nc.finalize()
    return nc


def _ptile(a2d, inner):
    """[R, C] row-major -> [P, R//P, C] partition-major, contiguous."""
    r, c = a2d.shape
    return np.ascontiguousarray(
        a2d.reshape(r // P, P, c).transpose(1, 0, 2)).astype(inner)


def _shard_inputs(inputs_q, inputs_kv, attention_mask, Wq, bq, Wk, bk, Wv, bv,
                  Wo, bo):
    bf16 = ml_dtypes.bfloat16
    f32 = np.float32

    xqT = [_ptile(inputs_q[b].T, bf16) for b in range(B)]         # [P,KC,TQ]
    xkvT = [_ptile(inputs_kv[b].T, bf16)                          # [P,NTC,KC,512]
            .reshape(P, KC, NTC, 512).transpose(0, 2, 1, 3).copy()
            for b in range(B)]
    maskT = [_ptile(attention_mask[b].T.astype(np.float32), bf16)  # [P,NTB,TQ]
             for b in range(B)]
    in_maps = []
    for c in range(NCORES):
        b, g = c // 2, c % 2  # pair = (2b, 2b+1)
        sl = slice(g * GD, (g + 1) * GD)
        in_maps.append({
            "xqT": xqT[b],
            "xkvT": xkvT[b],
            "maskT": maskT[b],
            "Wq": _ptile(np.ascontiguousarray(Wq[:, sl]), bf16),
            "Wk": _ptile(np.ascontiguousarray(Wk[:, sl]), bf16),
            "Wv": _ptile(np.ascontiguousarray(Wv[:, sl]), bf16),
            # all head rows x own col half, [P, H, GD] bf16
            "Wo": _ptile(np.ascontiguousarray(Wo[:, sl]), bf16),
            "bq": np.ascontiguousarray(
                bq[sl].reshape(GH, P).T).astype(f32),
            "bk": np.ascontiguousarray(
                bk[sl].reshape(GH, P).T).astype(f32),
            "bv": np.ascontiguousarray(bv[sl]).astype(f32),
            "bo": np.ascontiguousarray(
                bo[sl].reshape(NOB, P).T).astype(f32),
        })
    return in_maps


def kernel(_trace=False, **inputs):
    global _CACHED_NC
    from concourse import bass_utils

    arrs = {k: np.asarray(v) for k, v in inputs.items()}
    in_maps = _shard_inputs(**arrs)

    if _CACHED_NC is None:
        _CACHED_NC = _build_nc()

    res = bass_utils.run_bass_kernel_spmd(
        _CACHED_NC, in_maps, core_ids=list(range(NCORES)), trace=_trace)

    full = np.empty((B, TQ, D), np.float32)
    for c in range(NCORES):
        b, g = c // 2, c % 2
        o = res.results[c]["out"]  # [P, NOB, TQ] bf16, o-col = ob*128+p
        full[b][:, g * GD:(g + 1) * GD] = (
            o.transpose(2, 1, 0).reshape(TQ, GD).astype(np.float32))
    if _trace:
        return full, res
    return full
